# revision 1
# baseline (speedup 1.0000x reference)
"""GQA kernel for Trainium2, 8 NeuronCores.

Sharding: core c = b*4 + g  handles batch b, kv-head g (4 query heads).
Each core computes:
  Q_g^T = Wq_g @ x_q^T        [4 heads][128, S]   (scale 1/sqrt(D) folded in)
  K_g^T = Wk_g @ x_k^T        [128, S]
  V_g   = (x_v @ Wv_g.T)      [S, 128]  (via V^T then PE transpose)
  S^T   = K tile @ Q^T        [k,q] orientation -> +mask (diag) -> exp
  o^T  += V[kt] matmul P~^T   (PSUM accum), l += ones^T P~^T
  o_norm^T = o^T * recip(bcast l)
  partial = o_norm @ Wo_g.T   [S, E]
Host sums the 4 partials per batch.

Matmuls run in bf16 (fp32 PSUM accumulation): 4-byte dtypes serialize
LDWEIGHTS with the matmul (~191ns per 128x128 load, no FWL/prefetch),
which was ~37% of the kernel span in fp32r. l is broadcast across
partitions with a K=1 matmul so the reciprocal runs at full DVE lane
width ([128,512] not [1,512]).
"""

import sys

import numpy as np

for _p in ("/opt/trn_rl_repo",):
    if _p not in sys.path:
        sys.path.insert(0, _p)

import ml_dtypes

import concourse.bass as bass
import concourse.mybir as mybir
from concourse import bacc
from concourse.bass_utils import run_bass_kernel_spmd
from concourse.masks import make_identity
from concourse.tile import TileContext

B, S, E = 2, 2048, 2048
H, HKV = 16, 4
D = E // H  # 128
G = H // HKV  # 4 query heads per kv head
GD = G * D  # 512
NCORES = B * HKV  # 8
SC = 512  # s/q chunk width (free dim of matmuls)
NSC = S // SC  # 4
NET = E // 128  # 16 e-tiles (contraction)
NKT = S // 128  # 16 k-tiles
SCALE = 1.0 / float(np.sqrt(D))

F32 = mybir.dt.float32
BF16 = mybir.dt.bfloat16
F32R = mybir.dt.float32r
AF = mybir.ActivationFunctionType
NPBF = np.dtype(ml_dtypes.bfloat16)


def build_nc():
    nc = bacc.Bacc()
    xq = nc.declare_dram_parameter("xq", [E, S], BF16, isOutput=False)  # query[b].T
    xk = nc.declare_dram_parameter("xk", [E, S], BF16, isOutput=False)  # key[b].T
    xv = nc.declare_dram_parameter("xv", [E, S], BF16, isOutput=False)  # value[b].T
    wq = nc.declare_dram_parameter("wq", [E, GD], BF16, isOutput=False)
    wk = nc.declare_dram_parameter("wk", [E, D], BF16, isOutput=False)
    wv = nc.declare_dram_parameter("wv", [E, D], BF16, isOutput=False)
    wo = nc.declare_dram_parameter("wo", [GD, E], BF16, isOutput=False)
    msk = nc.declare_dram_parameter("msk", [4 * 128, SC], F32, isOutput=False)
    out = nc.declare_dram_parameter("out", [S, E], F32, isOutput=True)

    with TileContext(nc) as tc:
        with (
            tc.tile_pool(name="singles", bufs=1) as singles,
            tc.tile_pool(name="xt", bufs=24) as xtp,
            tc.tile_pool(name="pexp", bufs=4) as pexp,
            tc.tile_pool(name="small", bufs=2) as small,
            tc.tile_pool(name="ob", bufs=3) as obp,
            tc.tile_pool(name="acc", bufs=4, space="PSUM") as acc,
            tc.tile_pool(name="ops", bufs=2, space="PSUM") as ops,
            tc.tile_pool(name="lps", bufs=1, space="PSUM") as lps,
            tc.tile_pool(name="trp", bufs=1, space="PSUM") as trp,
            tc.tile_pool(name="drp", bufs=2, space="DRAM") as drp,
        ):
            # ---- constants / weights resident in SBUF ----
            wq_sb = singles.tile([128, NET, GD], BF16)  # 16KB/p
            wk_sb = singles.tile([128, NET, D], BF16)  # 4KB/p
            wv_sb = singles.tile([128, NET, D], BF16)  # 4KB/p
            wo_sb = singles.tile([128, G, E], BF16)  # 16KB/p
            mask_sb = singles.tile([128, 4, SC], F32)  # 8KB/p
            ident_f = singles.tile([128, 128], F32)
            ident = singles.tile([128, 128], BF16)
            ones_f = singles.tile([128, 1], F32)
            ones = singles.tile([128, 1], BF16)
            qT = singles.tile([128, G, S], BF16)  # 16KB/p
            kT = singles.tile([128, S], BF16)  # 4KB/p
            v_sb = singles.tile([128, NKT, D], BF16)  # 4KB/p
            onrm = singles.tile([128, G, S], BF16)  # 16KB/p
            o_unn = singles.tile([128, G, S], F32)  # 32KB/p

            make_identity(nc, ident_f)
            nc.scalar.activation(out=ident[:], in_=ident_f[:], func=AF.Copy)
            nc.vector.memset(ones_f, 1.0)
            nc.scalar.activation(out=ones[:], in_=ones_f[:], func=AF.Copy)
            for t in range(NET):
                nc.sync.dma_start(
                    out=wq_sb[:, t, :], in_=wq[t * 128 : (t + 1) * 128, :]
                )
                nc.sync.dma_start(out=wk_sb[:, t, :], in_=wk[t * 128 : (t + 1) * 128, :])
                nc.sync.dma_start(out=wv_sb[:, t, :], in_=wv[t * 128 : (t + 1) * 128, :])
            for h in range(G):
                nc.sync.dma_start(
                    out=wo_sb[:, h, :], in_=wo[h * 128 : (h + 1) * 128, :]
                )
            for j in range(4):
                nc.sync.dma_start(
                    out=mask_sb[:, j, :], in_=msk[j * 128 : (j + 1) * 128, :]
                )

            # ---- phase 1: projections ----
            for sc in range(NSC):
                ssl = slice(sc * SC, (sc + 1) * SC)
                # Q^T: 4 heads
                xts = []
                for t in range(NET):
                    xt = xtp.tile([128, SC], BF16, tag="xt")
                    nc.sync.dma_start(out=xt, in_=xq[t * 128 : (t + 1) * 128, ssl])
                    xts.append(xt)
                for h in range(G):
                    ps = acc.tile([128, SC], F32, tag="acc")
                    for t in range(NET):
                        nc.tensor.matmul(
                            ps[:],
                            lhsT=wq_sb[:, t, h * D : (h + 1) * D],
                            rhs=xts[t][:],
                            start=(t == 0),
                            stop=(t == NET - 1),
                        )
                    # fold softmax scale into Q
                    nc.scalar.activation(
                        out=qT[:, h, ssl], in_=ps[:], func=AF.Copy, scale=SCALE
                    )
                # K^T
                xts = []
                for t in range(NET):
                    xt = xtp.tile([128, SC], BF16, tag="xt")
                    nc.sync.dma_start(out=xt, in_=xk[t * 128 : (t + 1) * 128, ssl])
                    xts.append(xt)
                ps = acc.tile([128, SC], F32, tag="acc")
                for t in range(NET):
                    nc.tensor.matmul(
                        ps[:],
                        lhsT=wk_sb[:, t, :],
                        rhs=xts[t][:],
                        start=(t == 0),
                        stop=(t == NET - 1),
                    )
                nc.vector.tensor_copy(out=kT[:, ssl], in_=ps[:])
                # V^T then transpose to V [s, d]
                xts = []
                for t in range(NET):
                    xt = xtp.tile([128, SC], BF16, tag="xt")
                    nc.sync.dma_start(out=xt, in_=xv[t * 128 : (t + 1) * 128, ssl])
                    xts.append(xt)
                ps = acc.tile([128, SC], F32, tag="acc")
                for t in range(NET):
                    nc.tensor.matmul(
                        ps[:],
                        lhsT=wv_sb[:, t, :],
                        rhs=xts[t][:],
                        start=(t == 0),
                        stop=(t == NET - 1),
                    )
                vt_tmp = small.tile([128, SC], BF16, tag="vt")
                nc.scalar.activation(out=vt_tmp[:], in_=ps[:], func=AF.Copy)
                for i in range(SC // 128):
                    tp = trp.tile([128, 128], BF16, tag="tr")
                    nc.tensor.transpose(
                        tp[:], vt_tmp[:, i * 128 : (i + 1) * 128], ident[:]
                    )
                    nc.vector.tensor_copy(out=v_sb[:, sc * 4 + i, :], in_=tp[:])

            # ---- phase 2+3: attention, outproj interleaved per q-chunk ----
            for qc in range(NSC):
                for h in range(G):
                    qsl = slice(qc * SC, (qc + 1) * SC)
                    nkt = (qc + 1) * (SC // 128)  # causal: k tiles 0..nkt-1
                    o_ps = ops.tile([128, SC], F32, tag="o")
                    l_ps = lps.tile([1, SC], F32, tag="l")
                    for kt in range(nkt):
                        s_ps = acc.tile([128, SC], F32, tag="acc")
                        nc.tensor.matmul(
                            s_ps[:],
                            lhsT=kT[:, kt * 128 : (kt + 1) * 128],
                            rhs=qT[:, h, qsl],
                            start=True,
                            stop=True,
                        )
                        if kt >= nkt - 4:
                            j = kt - 4 * qc
                            nc.vector.tensor_add(s_ps[:], s_ps[:], mask_sb[:, j, :])
                        p_sb = pexp.tile([128, SC], BF16, tag="p")
                        nc.scalar.activation(out=p_sb[:], in_=s_ps[:], func=AF.Exp)
                        nc.tensor.matmul(
                            o_ps[:],
                            lhsT=v_sb[:, kt, :],
                            rhs=p_sb[:],
                            start=(kt == 0),
                            stop=(kt == nkt - 1),
                        )
                        nc.tensor.matmul(
                            l_ps[:],
                            lhsT=ones[:],
                            rhs=p_sb[:],
                            start=(kt == 0),
                            stop=(kt == nkt - 1),
                        )
                    # l broadcast across partitions via K=1 matmul, then
                    # reciprocal at full lane width and normalize.
                    nc.scalar.activation(
                        out=o_unn[:, h, qsl], in_=o_ps[:], func=AF.Copy
                    )
                    l_sb = small.tile([1, SC], F32, tag="lsb")
                    nc.scalar.activation(out=l_sb[:], in_=l_ps[:], func=AF.Copy)
                    l_dr = drp.tile([1, SC], F32, tag="ldr")
                    nc.sync.dma_start(out=l_dr[:], in_=l_sb[:])
                    lb = small.tile([128, SC], F32, tag="lb")
                    l_bc = bass.AP(
                        tensor=l_dr[:].tensor,
                        offset=l_dr[:].offset,
                        ap=[[0, 128]] + list(l_dr[:].ap[1:]),
                    )
                    nc.sync.dma_start(out=lb[:], in_=l_bc)
                    rb = small.tile([128, SC], F32, tag="rb")
                    nc.vector.reciprocal(out=rb[:], in_=lb[:])
                    nc.vector.tensor_mul(
                        onrm[:, h, qsl], o_unn[:, h, qsl], rb[:]
                    )

                # output projection for this q-chunk's 4 s-tiles
                for sti in range(SC // 128):
                    st = qc * (SC // 128) + sti
                    stl = slice(st * 128, (st + 1) * 128)
                    for ec in range(E // SC):
                        esl = slice(ec * SC, (ec + 1) * SC)
                        ps = acc.tile([128, SC], F32, tag="acc")
                        for h in range(G):
                            nc.tensor.matmul(
                                ps[:],
                                lhsT=onrm[:, h, stl],
                                rhs=wo_sb[:, h, esl],
                                start=(h == 0),
                                stop=(h == G - 1),
                            )
                        ob = obp.tile([128, SC], F32, tag="ob")
                        nc.scalar.activation(out=ob[:], in_=ps[:], func=AF.Copy)
                        nc.sync.dma_start(out=out[stl, esl], in_=ob[:])
    nc.compile()
    return nc


_NC_CACHE = None


def _get_nc():
    global _NC_CACHE
    if _NC_CACHE is None:
        _NC_CACHE = build_nc()
    return _NC_CACHE


def _prep_in_maps(query, key, value, attn_mask, Wq, Wk, Wv, Wo):
    query = np.asarray(query, dtype=np.float32)
    key = np.asarray(key, dtype=np.float32)
    value = np.asarray(value, dtype=np.float32)
    Wq = np.asarray(Wq, dtype=np.float32)
    Wk = np.asarray(Wk, dtype=np.float32)
    Wv = np.asarray(Wv, dtype=np.float32)
    Wo = np.asarray(Wo, dtype=np.float32)
    am = np.asarray(attn_mask)

    xqT = [np.ascontiguousarray(query[b].T).astype(NPBF) for b in range(B)]
    xkT = [np.ascontiguousarray(key[b].T).astype(NPBF) for b in range(B)]
    xvT = [np.ascontiguousarray(value[b].T).astype(NPBF) for b in range(B)]

    # 4 diagonal mask tiles [128, SC]: tile j covers k in [j*128,(j+1)*128)
    # relative to the q-chunk start; additive -1e9 on masked entries.
    m0 = np.asarray(am[0, 0, :SC, :SC], dtype=np.float32)  # [q, k] for chunk 0
    msk_tiles = np.zeros((4 * 128, SC), dtype=np.float32)
    for j in range(4):
        msk_tiles[j * 128 : (j + 1) * 128, :] = (
            m0[:, j * 128 : (j + 1) * 128].T - 1.0
        ) * 1e9
    in_maps = []
    for b in range(B):
        for g in range(HKV):
            in_maps.append(
                {
                    "xq": xqT[b],
                    "xk": xkT[b],
                    "xv": xvT[b],
                    "wq": np.ascontiguousarray(
                        Wq[g * GD : (g + 1) * GD, :].T
                    ).astype(NPBF),
                    "wk": np.ascontiguousarray(
                        Wk[g * D : (g + 1) * D, :].T
                    ).astype(NPBF),
                    "wv": np.ascontiguousarray(
                        Wv[g * D : (g + 1) * D, :].T
                    ).astype(NPBF),
                    "wo": np.ascontiguousarray(
                        Wo[:, g * GD : (g + 1) * GD].T
                    ).astype(NPBF),
                    "msk": msk_tiles,
                }
            )
    return in_maps


def _run(inputs, trace=False, **kw):
    nc = _get_nc()
    in_maps = _prep_in_maps(**inputs)
    res = run_bass_kernel_spmd(
        nc, in_maps, list(range(NCORES)), trace=trace, **kw
    )
    outs = [np.asarray(r["out"]) for r in res.results]
    full = np.empty((B, S, E), dtype=np.float32)
    for b in range(B):
        acc = outs[b * HKV].astype(np.float32)
        for g in range(1, HKV):
            acc = acc + outs[b * HKV + g]
        full[b] = acc
    return full, res


def kernel(**inputs):
    full, _ = _run(inputs, trace=False)
    return full



# revision 5
# speedup vs baseline: 1.4087x; 1.4087x over previous
"""GQA kernel for Trainium2, 8 NeuronCores.

Sharding: core c = b*4 + g handles batch b, kv-head g (4 query heads).
Each core computes (all matmuls bf16, fp32 PSUM):
  Q_g^T = Wq_g @ x_q^T   [4 heads][128, S]  (1/sqrt(D) folded into Wq host-side)
  K_g^T = Wk_g @ x_k^T   [128, S]
  V_g   = transpose(Wv_g @ x_v^T)           [S, 128] via PE transpose
  S^T   = K_kt^T Q       [k, q] -> +mask (diag tiles, DVE) -> exp (ACT)
  o^T  += V[kt] @ P~     (PSUM accum), l += ones^T P~
  o_norm^T = o^T * bcast(1/l)  (approx recip + SWDGE partition-broadcast)
  partial = o_norm @ Wo_g^T  [S, E]
Host sums the 4 partials per batch.

Perf structure (vs the naive version):
- All DRAM params are host-tiled to exactly match their SBUF layout, so
  every load is a big contiguous DMA (16KB rows).
- DMAs are split across both HW DGE queues (Sync + Scalar engines);
  out-writes also use the GpSimd SWDGE queue. The naive version pushed
  all 344 DMAs through the single Sync queue (~208us serialized).
- Attention is software-pipelined with a 4-tile score lookahead so the
  PE never waits on the score->exp->AV chain.
- Phases are interleaved (proj c0, att q0, proj c1, outproj q0, ...) so
  projection DMA latency hides under attention compute.
- Softmax normalization: reciprocal_approx_fast on [1,512] + SBUF->SBUF
  partition-broadcast DMA, replacing a DRAM round-trip + 3.3us full-width
  DVE reciprocal per head.
"""

import sys

import numpy as np

for _p in ("/opt/trn_rl_repo",):
    if _p not in sys.path:
        sys.path.insert(0, _p)

import ml_dtypes

import concourse.bass as bass
import concourse.mybir as mybir
from concourse import bacc
from concourse.bass_utils import run_bass_kernel_spmd
from concourse.masks import make_identity
from concourse.tile import TileContext

B, S, E = 2, 2048, 2048
H, HKV = 16, 4
D = E // H  # 128
G = H // HKV  # 4 query heads per kv head
GD = G * D  # 512
NCORES = B * HKV  # 8
SC = 512  # s/q chunk width (free dim of matmuls)
NSC = S // SC  # 4
NET = E // 128  # 16 e-tiles (contraction)
NKT = S // 128  # 16 k-tiles
SCALE = 1.0 / float(np.sqrt(D))

F32 = mybir.dt.float32
BF16 = mybir.dt.bfloat16
AF = mybir.ActivationFunctionType
NPBF = np.dtype(ml_dtypes.bfloat16)


def build_nc():
    nc = bacc.Bacc()
    # x tensors host-tiled: [p, chunk, etile, s2] so chunk loads are
    # contiguous 16KB rows per partition.
    xq = nc.declare_dram_parameter("xq", [128, NSC, NET, SC], BF16, isOutput=False)
    xk = nc.declare_dram_parameter("xk", [128, NSC, NET, SC], BF16, isOutput=False)
    xv = nc.declare_dram_parameter("xv", [128, NSC, NET, SC], BF16, isOutput=False)
    # weights host-tiled to SBUF layout
    wq = nc.declare_dram_parameter("wq", [128, NET, GD], BF16, isOutput=False)
    wk = nc.declare_dram_parameter("wk", [128, NET, D], BF16, isOutput=False)
    wv = nc.declare_dram_parameter("wv", [128, NET, D], BF16, isOutput=False)
    wo = nc.declare_dram_parameter("wo", [128, G, E], BF16, isOutput=False)
    msk = nc.declare_dram_parameter("msk", [128, 4, SC], F32, isOutput=False)
    out = nc.declare_dram_parameter("out", [S, E], F32, isOutput=True)

    with TileContext(nc) as tc:
        with (
            tc.tile_pool(name="singles", bufs=1) as singles,
            tc.tile_pool(name="xsp", bufs=5) as xsp,
            tc.tile_pool(name="pexp", bufs=6) as pexp,
            tc.tile_pool(name="vtp", bufs=2) as vtp,
            tc.tile_pool(name="ob", bufs=4) as obp,
            tc.tile_pool(name="rlp", bufs=2) as rlp,
            tc.tile_pool(name="rbp", bufs=2) as rbp,
            tc.tile_pool(name="drp", bufs=2, space="DRAM") as drp,
            tc.tile_pool(name="acc", bufs=4, space="PSUM") as acc,
            tc.tile_pool(name="ops", bufs=2, space="PSUM") as ops,
            tc.tile_pool(name="lps", bufs=1, space="PSUM") as lps,
            tc.tile_pool(name="trp", bufs=1, space="PSUM") as trp,
        ):
            # ---- SBUF-resident tensors ----
            wq_sb = singles.tile([128, NET, GD], BF16)  # 16KB/p
            wk_sb = singles.tile([128, NET, D], BF16)  # 4KB/p
            wv_sb = singles.tile([128, NET, D], BF16)  # 4KB/p
            wo_sb = singles.tile([128, G, E], BF16)  # 16KB/p
            mask_sb = singles.tile([128, 4, SC], F32)  # 8KB/p
            ident_f = singles.tile([128, 128], F32)
            ident = singles.tile([128, 128], BF16)
            ones_f = singles.tile([128, 1], F32)
            ones = singles.tile([128, 1], BF16)
            qT = singles.tile([128, G, S], BF16)  # 16KB/p
            kT = singles.tile([128, S], BF16)  # 4KB/p
            v_sb = singles.tile([128, NKT, D], BF16)  # 4KB/p
            onrm = singles.tile([128, G, S], BF16)  # 16KB/p

            make_identity(nc, ident_f)
            nc.scalar.activation(out=ident[:], in_=ident_f[:], func=AF.Copy)
            nc.vector.memset(ones_f, 1.0)
            nc.scalar.activation(out=ones[:], in_=ones_f[:], func=AF.Copy)

            xts = {}

            def load_x(c, order):
                tiles = []
                for nm, xx in (("q", xq), ("k", xk), ("v", xv)):
                    xt = xsp.tile([128, NET, SC], BF16, tag="x", name=f"x{nm}")
                    tiles.append((xt, xx))
                xts[c] = tuple(t for t, _ in tiles)
                for idx in order:
                    xt, xx = tiles[idx]
                    nc.sync.dma_start(out=xt[:, :8, :], in_=xx[:, c, :8, :])
                    nc.scalar.dma_start(out=xt[:, 8:, :], in_=xx[:, c, 8:, :])
                    if idx == 0 and c == 0:
                        # wq right behind xq-chunk0 on both queues
                        nc.sync.dma_start(out=wq_sb[:, :8, :], in_=wq[:, :8, :])
                        nc.scalar.dma_start(out=wq_sb[:, 8:, :], in_=wq[:, 8:, :])

            # ---- startup: chunk-0 x + early weights ----
            load_x(0, (0, 1, 2))
            nc.sync.dma_start(out=wv_sb[:], in_=wv[:])
            nc.scalar.dma_start(out=wk_sb[:], in_=wk[:])
            nc.scalar.dma_start(out=mask_sb[:], in_=msk[:])

            def load_wo():
                nc.sync.dma_start(out=wo_sb[:, :2, :], in_=wo[:, :2, :])
                nc.scalar.dma_start(out=wo_sb[:, 2:, :], in_=wo[:, 2:, :])

            def proj(c):
                ssl = slice(c * SC, (c + 1) * SC)
                xtq, xtk, xtv = xts[c]
                for h in range(G):
                    ps = acc.tile([128, SC], F32, tag="acc")
                    for t in range(NET):
                        nc.tensor.matmul(
                            ps[:],
                            lhsT=wq_sb[:, t, h * D : (h + 1) * D],
                            rhs=xtq[:, t, :],
                            start=(t == 0),
                            stop=(t == NET - 1),
                        )
                    nc.scalar.activation(out=qT[:, h, ssl], in_=ps[:], func=AF.Copy)
                ps = acc.tile([128, SC], F32, tag="acc")
                for t in range(NET):
                    nc.tensor.matmul(
                        ps[:],
                        lhsT=wk_sb[:, t, :],
                        rhs=xtk[:, t, :],
                        start=(t == 0),
                        stop=(t == NET - 1),
                    )
                nc.vector.tensor_copy(out=kT[:, ssl], in_=ps[:])
                ps = acc.tile([128, SC], F32, tag="acc")
                for t in range(NET):
                    nc.tensor.matmul(
                        ps[:],
                        lhsT=wv_sb[:, t, :],
                        rhs=xtv[:, t, :],
                        start=(t == 0),
                        stop=(t == NET - 1),
                    )
                vt = vtp.tile([128, SC], BF16, tag="vt")
                nc.scalar.activation(out=vt[:], in_=ps[:], func=AF.Copy)
                tp = trp.tile([128, 4, D], BF16, tag="tr")
                for i in range(4):
                    nc.tensor.transpose(
                        tp[:, i, :], vt[:, i * 128 : (i + 1) * 128], ident[:]
                    )
                nc.vector.tensor_copy(out=v_sb[:, c * 4 : (c + 1) * 4, :], in_=tp[:])

            def att(qc):
                qsl = slice(qc * SC, (qc + 1) * SC)
                nkt = 4 * (qc + 1)  # causal: k tiles 0..nkt-1
                work = [(h, kt) for h in range(G) for kt in range(nkt)]
                LA = 4  # score-tile lookahead
                ptiles = {}

                def issue_s(h, kt):
                    s_ps = acc.tile([128, SC], F32, tag="acc")
                    nc.tensor.matmul(
                        s_ps[:],
                        lhsT=kT[:, kt * 128 : (kt + 1) * 128],
                        rhs=qT[:, h, qsl],
                        start=True,
                        stop=True,
                    )
                    if kt >= nkt - 4:
                        nc.vector.tensor_add(
                            s_ps[:], s_ps[:], mask_sb[:, kt - 4 * qc, :]
                        )
                    p = pexp.tile([128, SC], BF16, tag="p")
                    nc.scalar.activation(out=p[:], in_=s_ps[:], func=AF.Exp)
                    ptiles[(h, kt)] = p

                for j in range(min(LA, len(work))):
                    issue_s(*work[j])
                otile = {}
                ltile = {}
                for i, (h, kt) in enumerate(work):
                    if kt == 0:
                        otile[h] = ops.tile([128, SC], F32, tag="o", name="o_ps")
                        ltile[h] = lps.tile([1, SC], F32, tag="l", name="l_ps")
                    if i + LA < len(work):
                        issue_s(*work[i + LA])
                    p = ptiles.pop((h, kt))
                    nc.tensor.matmul(
                        otile[h][:],
                        lhsT=v_sb[:, kt, :],
                        rhs=p[:],
                        start=(kt == 0),
                        stop=(kt == nkt - 1),
                    )
                    nc.tensor.matmul(
                        ltile[h][:],
                        lhsT=ones[:],
                        rhs=p[:],
                        start=(kt == 0),
                        stop=(kt == nkt - 1),
                    )
                    if kt == nkt - 1:
                        rl = rlp.tile([1, SC], F32, tag="rl")
                        nc.vector.reciprocal_approx_fast(out=rl[:], in_=ltile[h][:])
                        rd = drp.tile([1, SC], F32, tag="rd")
                        nc.gpsimd.dma_start(out=rd[:], in_=rl[:])
                        rb = rbp.tile([128, SC], F32, tag="rb")
                        rd_ap = rd[:]
                        rd_bc = bass.AP(
                            tensor=rd_ap.tensor,
                            offset=rd_ap.offset,
                            ap=[[0, 128]] + list(rd_ap.ap[1:]),
                        )
                        nc.gpsimd.dma_start(out=rb[:], in_=rd_bc)
                        nc.vector.tensor_mul(onrm[:, h, qsl], otile[h][:], rb[:])

            def outproj(qc):
                for sti in range(4):
                    st = qc * 4 + sti
                    stl = slice(st * 128, (st + 1) * 128)
                    for ec in range(E // SC):
                        esl = slice(ec * SC, (ec + 1) * SC)
                        ps = acc.tile([128, SC], F32, tag="acc")
                        for h in range(G):
                            nc.tensor.matmul(
                                ps[:],
                                lhsT=onrm[:, h, stl],
                                rhs=wo_sb[:, h, esl],
                                start=(h == 0),
                                stop=(h == G - 1),
                            )
                        ob = obp.tile([128, SC], F32, tag="ob")
                        nc.vector.tensor_copy(out=ob[:], in_=ps[:])
                        eng = nc.sync if (sti + ec) % 2 == 0 else nc.gpsimd
                        eng.dma_start(out=out[stl, esl], in_=ob[:])

            # ---- interleaved schedule ----
            proj(0)
            load_x(1, (0, 1, 2))
            load_wo()
            att(0)
            proj(1)
            load_x(2, (0, 1, 2))
            outproj(0)
            att(1)
            proj(2)
            load_x(3, (0, 1, 2))
            outproj(1)
            att(2)
            proj(3)
            outproj(2)
            att(3)
            outproj(3)
    nc.compile()
    return nc


_NC_CACHE = None


def _get_nc():
    global _NC_CACHE
    if _NC_CACHE is None:
        _NC_CACHE = build_nc()
    return _NC_CACHE


def _tile_x(xT):
    # xT: [E, S] f32 -> [128, NSC, NET, SC] bf16 (p, chunk, etile, s2)
    return np.ascontiguousarray(
        xT.reshape(NET, 128, NSC, SC).transpose(1, 2, 0, 3)
    ).astype(NPBF)


def _prep_in_maps(query, key, value, attn_mask, Wq, Wk, Wv, Wo):
    query = np.asarray(query, dtype=np.float32)
    key = np.asarray(key, dtype=np.float32)
    value = np.asarray(value, dtype=np.float32)
    Wq = np.asarray(Wq, dtype=np.float32)
    Wk = np.asarray(Wk, dtype=np.float32)
    Wv = np.asarray(Wv, dtype=np.float32)
    Wo = np.asarray(Wo, dtype=np.float32)
    am = np.asarray(attn_mask)

    xqs = [_tile_x(query[b].T) for b in range(B)]
    xks = [_tile_x(key[b].T) for b in range(B)]
    xvs = [_tile_x(value[b].T) for b in range(B)]

    # 4 diagonal mask tiles, [128(k), 4(j), SC(q)]: tile j covers k in
    # [j*128,(j+1)*128) relative to the q-chunk start; additive -1e9.
    m0 = np.asarray(am[0, 0, :SC, :SC], dtype=np.float32)  # [q, k] chunk 0
    msk_tiles = np.empty((128, 4, SC), dtype=np.float32)
    for j in range(4):
        msk_tiles[:, j, :] = (m0[:, j * 128 : (j + 1) * 128].T - 1.0) * 1e9

    def tile_w(wT, width):
        # wT: [E, width] -> [128, NET, width]
        return np.ascontiguousarray(
            wT.reshape(NET, 128, width).transpose(1, 0, 2)
        ).astype(NPBF)

    in_maps = []
    for b in range(B):
        for g in range(HKV):
            wqT = Wq[g * GD : (g + 1) * GD, :].T * SCALE  # fold softmax scale
            wkT = Wk[g * D : (g + 1) * D, :].T
            wvT = Wv[g * D : (g + 1) * D, :].T
            woT = Wo[:, g * GD : (g + 1) * GD].T  # [GD, E]
            in_maps.append(
                {
                    "xq": xqs[b],
                    "xk": xks[b],
                    "xv": xvs[b],
                    "wq": tile_w(wqT, GD),
                    "wk": tile_w(wkT, D),
                    "wv": tile_w(wvT, D),
                    "wo": np.ascontiguousarray(
                        woT.reshape(G, 128, E).transpose(1, 0, 2)
                    ).astype(NPBF),
                    "msk": msk_tiles,
                }
            )
    return in_maps


def _run(inputs, trace=False, **kw):
    nc = _get_nc()
    in_maps = _prep_in_maps(**inputs)
    res = run_bass_kernel_spmd(nc, in_maps, list(range(NCORES)), trace=trace, **kw)
    outs = [np.asarray(r["out"]) for r in res.results]
    full = np.empty((B, S, E), dtype=np.float32)
    for b in range(B):
        acc = outs[b * HKV].astype(np.float32)
        for g in range(1, HKV):
            acc = acc + outs[b * HKV + g]
        full[b] = acc
    return full, res


def kernel(**inputs):
    full, _ = _run(inputs, trace=False)
    return full
